# revision 10
# baseline (speedup 1.0000x reference)
"""EnhancedLoRALinear Trainium2 kernel.

Computes, for x:[4,8192,1024] and torch-style weights (out,in):
    out = x @ (W + W_res)^T + b + sigmoid(x @ W_gate^T) * (2 * (x @ W_down^T) @ W_up^T)

Strategy (v4):
  - Data-parallel: the 32768 tokens are split across 8 NeuronCores (4096 each);
    the small weight matrices are replicated.
  - Algebraic fold: main + residual share one matmul with Wc = W + W_res.
  - Dtypes by accuracy need (tolerance 2e-2, measured ~4e-3):
      main path  : bf16 x / bf16 Wc           (full-rate MM, FWL on LDWEIGHTS)
      gate path  : fp8e4 x / fp8e4 64*W_gate  (DoubleRow: 2 k-tiles per MM,
                   the x64 weight scale keeps values out of fp8 subnormals;
                   undone for free via sigmoid's scale=1/64)
      down path  : fp8e4 DoubleRow with a widened W_down that also emits a
                   copy of the down-projection on partitions 32:48, so the
                   up-projection for both output halves runs as two
                   concurrent row-tiled (tile_position) matmuls
      up path    : bf16 with (2/64)*W_up
  - All tensors are host-swizzled into SBUF layout so every DMA moves
    per-partition-contiguous bytes (full HBM bandwidth, few descriptors).
  - Prologue: DMAs are issued in PE consumption order (wd2, x8/xb chunks of
    group 0 interleaved with Wc k-pairs, bias/W_up early, W_gate halves) so
    real matmuls start as soon as the first operands land. A short junk spin
    covers engine startup for the HAM clock gate.
  - Matmuls per 128-token tile are interleaved over the two 512-wide output
    halves (k-outer), giving stationary loads a two-matmul hiding window.
  - Epilogue: VectorE drains mps early (m2 = mps + bias; GpSimd cannot read
    PSUM), ScalarE does the sigmoid, VectorE the gate*lora, GpSimd the final
    SBUF-only add (VectorE for the last tile to shorten the tail).
"""

import ml_dtypes
import numpy as np

_BF16 = ml_dtypes.bfloat16
_F8 = ml_dtypes.float8_e4m3

import concourse.bass as bass
import concourse.bacc as bacc
import concourse.mybir as mybir
import concourse.tile as tile
from concourse.bass_utils import run_bass_kernel_spmd
from concourse.tile_rust import add_dep_helper

N_CORES = 8
B, S = 4, 8192
TOK = B * S                  # 32768 tokens total
T = TOK // N_CORES           # 4096 tokens per core
I = 1024                     # in_features
O = 1024                     # out_features
R = 16                       # lora rank
KT = I // 128                # 8 contraction tiles
TG = 512                     # token group (down-projection batch)
NG = T // TG                 # 8 groups per core
NH = O // 512                # 2 output halves
WS = 64.0                    # fp8 weight pre-scale (power of two, exact)
RW = 48                      # widened down-projection rows (16 + 16 zero + 16)

F32 = mybir.dt.float32
BF16 = mybir.dt.bfloat16
F8E4 = mybir.dt.float8e4
DR = mybir.MatmulPerfMode.DoubleRow


def _build_nc():
    nc = bacc.Bacc(None)

    # all inputs pre-swizzled to SBUF layout (partition-contiguous lines)
    xb = nc.dram_tensor("xb", [128, NG * KT, TG], BF16, kind="ExternalInput")
    x8 = nc.dram_tensor("x8", [128, NG * KT, TG], F8E4, kind="ExternalInput")
    wcb = nc.dram_tensor("wcb", [128, KT, O], BF16, kind="ExternalInput")
    wg8 = nc.dram_tensor("wg8", [128, KT, O], F8E4, kind="ExternalInput")
    wd2 = nc.dram_tensor("wd2", [128, KT, RW], F8E4, kind="ExternalInput")
    wu2 = nc.dram_tensor("wu2", [64, 512], BF16, kind="ExternalInput")
    biasbc = nc.dram_tensor("biasbc", [128, O], F32, kind="ExternalInput")
    out = nc.dram_tensor("out", [T, O], F32, kind="ExternalOutput")

    sig = mybir.ActivationFunctionType.Sigmoid
    mult = mybir.AluOpType.mult
    add = mybir.AluOpType.add

    with tile.TileContext(nc) as tc:
        with (
            tc.tile_pool(name="wpool", bufs=1) as wpool,
            tc.tile_pool(name="xpool", bufs=3) as xpool,
            tc.tile_pool(name="opool", bufs=3) as opool,
            tc.tile_pool(name="epool", bufs=3) as epool,
            tc.tile_pool(name="psum", bufs=1, space="PSUM") as pp,
        ):
            # --- resident weights ---
            wc_sb = wpool.tile([128, KT, O], BF16)
            wg_sb = wpool.tile([128, KT, O], F8E4)
            wd_sb = wpool.tile([128, KT, RW], F8E4)
            wu_sb = wpool.tile([64, 512], BF16)
            bias_bc = wpool.tile([128, O], F32)

            # group-0 x tiles, hoisted so their DMAs issue early
            xb0_sb = xpool.tile([128, KT, TG], BF16, tag="xb")
            x80_sb = xpool.tile([128, KT, TG], F8E4, tag="x8")

            # DMA issue order = PE consumption order (group 0 pipelined
            # through the single FIFO HWDGE ring)
            nc.sync.dma_start(out=wd_sb[:, :, :], in_=wd2[:, :, :])
            nc.sync.dma_start(out=x80_sb[:, :, :], in_=x8[:, 0:KT, :])
            nc.sync.dma_start(out=xb0_sb[:, 0:2, :], in_=xb[:, 0:2, :])
            nc.sync.dma_start(out=wc_sb[:, 0:2, :], in_=wcb[:, 0:2, :])
            nc.sync.dma_start(out=xb0_sb[:, 2:4, :], in_=xb[:, 2:4, :])
            nc.sync.dma_start(out=wc_sb[:, 2:4, :], in_=wcb[:, 2:4, :])
            nc.sync.dma_start(out=wg_sb[:, 0:4, :], in_=wg8[:, 0:4, :])
            nc.sync.dma_start(out=xb0_sb[:, 4:6, :], in_=xb[:, 4:6, :])
            nc.sync.dma_start(out=wc_sb[:, 4:6, :], in_=wcb[:, 4:6, :])
            nc.sync.dma_start(out=xb0_sb[:, 6:8, :], in_=xb[:, 6:8, :])
            nc.sync.dma_start(out=wc_sb[:, 6:8, :], in_=wcb[:, 6:8, :])
            nc.sync.dma_start(out=wg_sb[:, 4:8, :], in_=wg8[:, 4:8, :])
            nc.sync.dma_start(out=bias_bc[:, :], in_=biasbc[:, :])
            nc.sync.dma_start(out=wu_sb[:, :], in_=wu2[:, :])

            # HAM spin-up: a short junk spin covers engine startup until the
            # first DMAs land; real matmuls keep the PE busy from then on
            junk = wpool.tile([128, 512], BF16)
            nc.vector.memset(junk[:, :], 0.0)
            warm = pp.tile([128, 512], F32, tag="warm")
            spin = None
            for i in range(4):
                spin = nc.tensor.matmul(warm[:, :], junk[:, 0:128], junk[:, :],
                                        start=True, stop=True)

            # warm-up matmuls observing the weight-DMA semaphores of the two
            # chains that would otherwise need two semaphores at once
            warm_wd = nc.tensor.matmul(warm[0:RW, 0:RW], wd_sb[:, 0, :],
                                       wd_sb[:, 0, :], start=True, stop=True)
            warm_wc = nc.tensor.matmul(warm[0:1, 0:128], wc_sb[:, 0, 0:1],
                                       wc_sb[:, 0, 0:128], start=True,
                                       stop=True)
            warm_deps = {"down": [warm_wd, spin], "main": [warm_wc, spin]}
            first_real = {"down": [], "main": []}

            for g in range(NG):
                tg0 = g * TG
                if g == 0:
                    xb_sb, x8_sb = xb0_sb, x80_sb
                else:
                    xb_sb = xpool.tile([128, KT, TG], BF16, tag="xb")
                    nc.sync.dma_start(
                        out=xb_sb[:, :, :],
                        in_=xb[:, g * KT : (g + 1) * KT, :],
                    )
                    x8_sb = xpool.tile([128, KT, TG], F8E4, tag="x8")
                    nc.sync.dma_start(
                        out=x8_sb[:, :, :],
                        in_=x8[:, g * KT : (g + 1) * KT, :],
                    )

                # LoRA down-projection for the whole 512-token group,
                # [RW, TG]: rows 0:16 = 64*down, 16:32 zero, 32:48 = 64*down
                dps = pp.tile([RW, TG], F32, tag="misc")
                nc.scalar.memzero(dps[:, :])
                for kk in range(KT // 2):
                    mm = nc.tensor.matmul(
                        dps[:, :],
                        wd_sb[:, 2 * kk : 2 * kk + 2, :],
                        x8_sb[:, 2 * kk : 2 * kk + 2, :],
                        start=False,
                        stop=(kk == KT // 2 - 1),
                        skip_group_check=True,
                        perf_mode=DR,
                    )
                    if g == 0 and kk == 0:
                        first_real["down"].append(mm)
                down_sb = epool.tile([RW, TG], BF16, tag="down")
                nc.vector.tensor_copy(down_sb[:, :], dps[:, :])

                for t in range(TG // 128):
                    tsl = slice(t * 128, (t + 1) * 128)
                    last_tile = g == NG - 1 and t == TG // 128 - 1
                    out_sb = opool.tile([128, O], F32, tag="out")
                    osl = [slice(oh * 512, (oh + 1) * 512) for oh in range(NH)]
                    mps = [pp.tile([128, 512], F32, tag=f"main{oh}",
                                   name=f"mps{oh}") for oh in range(NH)]
                    gps = [pp.tile([128, 512], F32, tag=f"gate{oh}",
                                   name=f"gps{oh}") for oh in range(NH)]
                    lps = [pp.tile([128, 512], F32, tag=f"lora{oh}",
                                   name=f"lps{oh}") for oh in range(NH)]
                    for k in range(KT):
                        for oh in range(NH):
                            mm = nc.tensor.matmul(
                                mps[oh][:, :],
                                xb_sb[:, k, tsl],
                                wc_sb[:, k, osl[oh]],
                                start=(k == 0),
                                stop=(k == KT - 1),
                            )
                            if g == 0 and t == 0 and k == 0 and oh == 0:
                                first_real["main"].append(mm)
                    # up-projection first (its operands are ready at group
                    # start, and its LDWEIGHTS hide under the main chain):
                    # two concurrent row-tiled matmuls (the stationary/moving
                    # APs at base partition 32*oh derive tile_position row
                    # groups 0 and 1)
                    for oh in range(NH):
                        nc.tensor.matmul(
                            lps[oh][:, :],
                            down_sb[32 * oh : 32 * oh + R, tsl],
                            wu_sb[32 * oh : 32 * oh + R, :],
                            start=True,
                            stop=True,
                        )
                    for oh in range(NH):
                        nc.scalar.memzero(gps[oh][:, :])
                    for kk in range(KT // 2):
                        for oh in range(NH):
                            nc.tensor.matmul(
                                gps[oh][:, :],
                                x8_sb[:, 2 * kk : 2 * kk + 2, tsl],
                                wg_sb[:, 2 * kk : 2 * kk + 2, osl[oh]],
                                start=False,
                                stop=(kk == KT // 2 - 1),
                                skip_group_check=True,
                                perf_mode=DR,
                            )
                    for oh in range(NH):
                        # epilogue: VectorE drains mps early (GpSimd cannot
                        # read PSUM) while ScalarE computes the sigmoid
                        # (whose scale undoes the fp8 x64 weight pre-scale);
                        # the final SBUF-only add goes to GpSimd
                        m2_sb = epool.tile([128, 512], F32, tag=f"m2{oh}")
                        nc.vector.tensor_tensor(
                            m2_sb[:, :], mps[oh][:, :], bias_bc[:, osl[oh]],
                            add
                        )
                        g_sb = epool.tile([128, 512], F32, tag=f"sig{oh}")
                        nc.scalar.activation(
                            g_sb[:, :], gps[oh][:, :], sig, scale=1.0 / WS
                        )
                        gl_sb = epool.tile([128, 512], F32, tag=f"gl{oh}")
                        nc.vector.tensor_tensor(
                            gl_sb[:, :], g_sb[:, :], lps[oh][:, :], mult
                        )
                        adder = nc.vector if last_tile else nc.gpsimd
                        adder.tensor_tensor(
                            out_sb[:, osl[oh]], gl_sb[:, :], m2_sb[:, :], add
                        )
                        if last_tile:
                            nc.sync.dma_start(
                                out=out[tg0 + t * 128 : tg0 + (t + 1) * 128,
                                        osl[oh]],
                                in_=out_sb[:, osl[oh]],
                            )
                    if not last_tile:
                        nc.sync.dma_start(
                            out=out[tg0 + t * 128 : tg0 + (t + 1) * 128, :],
                            in_=out_sb[:, :],
                        )

            # ordering-only deps: each warm-up precedes only the chain that
            # needs its weight-DMA semaphore, so the chains' hw sync-wait
            # slot stays free for their x-tile DMA semaphore
            for key, ws in warm_deps.items():
                for w in ws:
                    for fr in first_real[key]:
                        add_dep_helper(fr.ins, w.ins, False,
                                       "warmups before real matmuls")
    nc.compile()
    return nc


_NC_CACHE = None


def _get_nc():
    global _NC_CACHE
    if _NC_CACHE is None:
        _NC_CACHE = _build_nc()
    return _NC_CACHE


def _swz(a, free):
    """[I, F] -> [128, KT, F] partition-contiguous swizzle."""
    return np.ascontiguousarray(a.reshape(KT, 128, free).transpose(1, 0, 2))


def _prep_inputs(x, W, b, W_down, W_up, W_gate, W_res):
    x = np.asarray(x, dtype=np.float32).reshape(TOK, I)
    wcb = _swz((np.asarray(W) + np.asarray(W_res)).T.astype(_BF16), O)
    wg8 = _swz((WS * np.asarray(W_gate)).T.astype(_F8), O)
    # widened down weights: columns 0:16 = 64*Wd^T, 16:32 = 0, 32:48 = 64*Wd^T
    wdt = (WS * np.asarray(W_down)).T.astype(_F8)          # [I, R]
    wd2 = np.zeros((I, RW), dtype=_F8)
    wd2[:, 0:R] = wdt
    wd2[:, 2 * R : 3 * R] = wdt
    wd2 = _swz(wd2, RW)
    # packed up weights: rows 0:16 -> half 0, rows 32:48 -> half 1
    wut = ((2.0 / WS) * np.asarray(W_up)).T.astype(_BF16)  # [R, O]
    wu2 = np.zeros((64, 512), dtype=_BF16)
    wu2[0:R, :] = wut[:, 0:512]
    wu2[2 * R : 3 * R, :] = wut[:, 512:1024]
    biasbc = np.ascontiguousarray(
        np.broadcast_to(np.asarray(b, dtype=np.float32).reshape(1, O), (128, O))
    )
    in_maps = []
    for c in range(N_CORES):
        xt_c = np.ascontiguousarray(x[c * T : (c + 1) * T, :].T)  # [I, T]
        # [I, T] -> [128, NG*KT, TG]: per-group-per-ktile contiguous lines
        xs = xt_c.reshape(KT, 128, NG, TG).transpose(1, 2, 0, 3)
        xs = np.ascontiguousarray(xs).reshape(128, NG * KT, TG)
        in_maps.append(
            {
                "xb": xs.astype(_BF16),
                "x8": xs.astype(_F8),
                "wcb": wcb,
                "wg8": wg8,
                "wd2": wd2,
                "wu2": wu2,
                "biasbc": biasbc,
            }
        )
    return in_maps


def run(inputs, trace=False, **kwargs):
    """Build + run on the 8 NeuronCores. Returns (full_output, BassKernelResults)."""
    nc = _get_nc()
    in_maps = _prep_inputs(**inputs)
    res = run_bass_kernel_spmd(
        nc, in_maps, list(range(N_CORES)), trace=trace, **kwargs
    )
    shards = [res.results[c]["out"] for c in range(N_CORES)]
    full = np.concatenate(shards, axis=0).reshape(B, S, O)
    return full, res


def kernel(**inputs):
    out, _ = run(inputs, trace=False)
    return out
